# revision 16
# baseline (speedup 1.0000x reference)
"""DAGNN K-hop propagation (out = sum_k softmax(att)[k] * A^k x) on 8 TRN2 cores.

Sharding: nodes are split into 8 contiguous shards of 6272 rows (49 blocks of
128). Edges are partitioned by destination-owner core, so each core's
segment-sum is local; per hop, the 8 bf16 h-shards are exchanged with two
AllGathers (split by local row range so each gather window fits int16
indices).

v2 restructure (Q7-saturating schedule): the GpSimd engine (which serially
generates every gather's DMA descriptors) is the bottleneck, so the per-hop
instruction stream is ordered to keep it busy end-to-end:
  - B-half gathers lag A-half gathers by LAG chunks, so the hop's first
    gathers (A, reading the buf0 window) never wait on the late AllGather
    (buf1), and the AG1 collective instruction - which runs on the GpSimd
    queue and blocks it while waiting for its staging DMA - is issued early
    in the NEXT hop when its input is already staged.
  - AG0 covers blocks 0..23 (staged mid-hop after chunk 3's results), its
    collective issued late in the hop when staging has completed.
  - PSUM->SBUF h-staging copies run on the idle Scalar/ACT engine; out_acc
    updates are a single fused DVE op.

Per hop, per core: dma_gather pulls the ~88k source-row slots (bf16, 256B
each) from DRAM into SBUF in 128-edge tiles laid out per (dst-block,
src-window) segment; a one-hot segment matrix S per tile (broadcast-AP
vector-engine is_equal per chunk-half) is matmul'd with the gathered tile on
the tensor engine, accumulating each dst-block's segment sum in PSUM. Pad
slots gather row 0 with dst value -1, which maps to a zero one-hot column.
"""
import os
import sys

sys.path.insert(0, "/opt/trn_rl_repo")

import numpy as np
import ml_dtypes

import concourse.bacc as bacc
import concourse.mybir as mybir
from concourse import tile
from concourse.bass_utils import run_bass_kernel_spmd

N_NODES, N_EDGES, D, K = 50000, 625000, 128, 10
CORES, NB = 8, 49
NPC = NB * 128              # 6272 nodes per core
PAD_N = CORES * NPC         # 50176
SPLIT_L = 3200              # local-row split: rows [0,3200) -> buf0
NB0 = SPLIT_L // 128        # 24 blocks in buf0 half
WIN0 = CORES * SPLIT_L      # 24576 rows in buf0
WIN1 = PAD_N - WIN0         # 25600 rows in buf1
LPC1 = NPC - SPLIT_L        # 3200 rows/core in buf1
CH_SIZES = [5] * 9 + [4]
CH_STARTS = [5 * i for i in range(9)] + [45]
NCH = len(CH_SIZES)
NCH0 = 5                    # chunks 0..4 cover buf0's blocks (0..24)
BF16 = ml_dtypes.bfloat16

LAST_EXEC_NS = None         # set when BASS_KERNEL_TRACE=1
LAST_RESULT = None

_nc_cache = {}


def _host_prep(x, att, edge_index):
    src = np.asarray(edge_index[0], dtype=np.int64)
    dst = np.asarray(edge_index[1], dtype=np.int64)
    # src row position in the split h-table layout
    c0 = src // NPC
    l0 = src % NPC
    half = (l0 >= SPLIT_L).astype(np.int64)
    rowidx = np.where(half == 0, c0 * SPLIT_L + l0,
                      c0 * LPC1 + (l0 - SPLIT_L))
    gblk = dst >> 7                                  # global 128-block, 0..390
    seg = gblk * 2 + half
    order = np.argsort(seg, kind="stable")
    seg_s = seg[order]
    counts = np.bincount(seg, minlength=2 * (NB * CORES))
    T_A = int(np.ceil(counts[0::2].max() / 128))
    T_B = int(np.ceil(counts[1::2].max() / 128))
    LA, LB = NB * T_A * 128, NB * T_B * 128

    starts = np.zeros(2 * NB * CORES + 1, np.int64)
    np.cumsum(counts, out=starts[1:])
    rank = np.arange(N_EDGES) - starts[seg_s]
    row_s, dst_s = rowidx[order], dst[order]
    core = gblk[order] // NB
    b = gblk[order] % NB
    mA = seg_s % 2 == 0

    idxA = np.full((CORES, LA), 0, np.int16)
    dvA = np.full((CORES, LA), -1.0, np.float32)
    idxB = np.full((CORES, LB), 0, np.int16)
    dvB = np.full((CORES, LB), -1.0, np.float32)
    sA = b[mA] * (T_A * 128) + rank[mA]
    idxA[core[mA], sA] = row_s[mA].astype(np.int16)
    dvA[core[mA], sA] = (dst_s[mA] & 127).astype(np.float32)
    mB = ~mA
    sB = b[mB] * (T_B * 128) + rank[mB]
    idxB[core[mB], sB] = row_s[mB].astype(np.int16)
    dvB[core[mB], sB] = (dst_s[mB] & 127).astype(np.float32)

    # h0 in the split layout
    x_pad = np.zeros((PAD_N, D), np.float32)
    x_pad[:N_NODES] = np.asarray(x, np.float32)
    xr = x_pad.reshape(CORES, NPC, D)
    h0 = np.concatenate(
        [xr[:, :SPLIT_L].reshape(-1, D), xr[:, SPLIT_L:].reshape(-1, D)],
        axis=0).astype(BF16)
    attw = np.ascontiguousarray(att, dtype=np.float32).reshape(1, K + 1)

    in_maps = []
    for c in range(CORES):
        in_maps.append({
            "h0": h0,
            "xsh": np.ascontiguousarray(x_pad[c * NPC:(c + 1) * NPC]),
            "attw": attw,
            # idx i lives at [i % 16, i // 16], replicated to 8 groups of 16
            # partitions for the Q7 cores.
            "idxA": np.ascontiguousarray(np.tile(idxA[c].reshape(-1, 16).T, (8, 1))),
            "idxB": np.ascontiguousarray(np.tile(idxB[c].reshape(-1, 16).T, (8, 1))),
            # dstv column t holds tile t's per-edge dst-within-block values.
            "dvA": np.ascontiguousarray(dvA[c].reshape(-1, 128).T),
            "dvB": np.ascontiguousarray(dvB[c].reshape(-1, 128).T),
        })
    return T_A, T_B, in_maps


def _build(T_A, T_B, n_iters=K):
    dt = mybir.dt
    nc = bacc.Bacc("TRN2", target_bir_lowering=False, debug=False,
                   num_devices=CORES, num_swdge_queues=4)
    h0 = nc.dram_tensor("h0", [PAD_N, D], dt.bfloat16, kind="ExternalInput")
    xsh = nc.dram_tensor("xsh", [NPC, D], dt.float32, kind="ExternalInput")
    attw = nc.dram_tensor("attw", [1, K + 1], dt.float32, kind="ExternalInput")
    idxA = nc.dram_tensor("idxA", [128, NB * T_A * 8], dt.int16, kind="ExternalInput")
    idxB = nc.dram_tensor("idxB", [128, NB * T_B * 8], dt.int16, kind="ExternalInput")
    dvA = nc.dram_tensor("dvA", [128, NB * T_A], dt.float32, kind="ExternalInput")
    dvB = nc.dram_tensor("dvB", [128, NB * T_B], dt.float32, kind="ExternalInput")
    outp = nc.dram_tensor("out", [NPC, D], dt.float32, kind="ExternalOutput")
    iota_c = nc.inline_tensor(
        np.ascontiguousarray(
            np.broadcast_to(np.arange(128, dtype=np.float32), (128, 128))
        ),
        name="iota",
    )

    with tile.TileContext(nc) as tc:
        with (
            tc.tile_pool(name="pers", bufs=1) as pers,
            tc.tile_pool(name="gapool", bufs=5) as gapool,
            tc.tile_pool(name="gbpool", bufs=7) as gbpool,
            tc.tile_pool(name="spool", bufs=2) as spool,
            tc.tile_pool(name="pp", bufs=4, space="PSUM") as pp,
            tc.tile_pool(name="tpool", bufs=2) as tpool,
            tc.tile_pool(name="dram", bufs=2, space="DRAM") as dram,
        ):
            idxA_s = pers.tile([128, NB * T_A * 8], dt.int16)
            nc.sync.dma_start(idxA_s[:], idxA[:])
            idxB_s = pers.tile([128, NB * T_B * 8], dt.int16)
            nc.sync.dma_start(idxB_s[:], idxB[:])
            dvA_s = pers.tile([128, NB * T_A], dt.float32)
            nc.sync.dma_start(dvA_s[:], dvA[:])
            dvB_s = pers.tile([128, NB * T_B], dt.float32)
            nc.sync.dma_start(dvB_s[:], dvB[:])
            iota_s = pers.tile([128, 128], dt.float32)
            nc.sync.dma_start(iota_s[:], iota_c[:])

            # w = softmax(att) on partition 0, broadcast to all partitions.
            att_s = pers.tile([1, K + 1], dt.float32)
            nc.sync.dma_start(att_s[:], attw[:])
            wexp = pers.tile([1, K + 1], dt.float32)
            nc.scalar.activation(wexp[:], att_s[:],
                                 mybir.ActivationFunctionType.Exp)
            wsum = pers.tile([1, 1], dt.float32)
            nc.vector.tensor_reduce(wsum[:], wexp[:], mybir.AxisListType.X,
                                    mybir.AluOpType.add)
            wrec = pers.tile([1, 1], dt.float32)
            nc.vector.reciprocal(wrec[:], wsum[:])
            wnorm = pers.tile([1, K + 1], dt.float32)
            nc.vector.tensor_scalar_mul(wnorm[:], wexp[:], wrec[:])
            wb = pers.tile([128, K + 1], dt.float32)
            nc.gpsimd.partition_broadcast(wb[:], wnorm[:])

            # out_acc[p, b*D:...] accumulates node (b*128+p)'s output row.
            out_acc = pers.tile([128, NB * D], dt.float32)
            hstage = pers.tile([128, NB * D], dt.bfloat16)
            for ci in range(NCH):
                g0, nbg = CH_STARTS[ci], CH_SIZES[ci]
                xc = tpool.tile([128, nbg * D], dt.float32, tag="tmp")
                nc.sync.dma_start(
                    xc[:].rearrange("p (b f) -> p b f", f=D),
                    xsh.ap()[g0 * 128:(g0 + nbg) * 128, :]
                    .rearrange("(b p) f -> p b f", p=128))
                # init on the ACT engine: out_acc = w0 * x
                nc.scalar.activation(
                    out_acc[:, g0 * D:(g0 + nbg) * D], xc[:],
                    mybir.ActivationFunctionType.Copy, scale=wb[:, 0:1])

            srcs = (h0.ap()[0:WIN0, :], h0.ap()[WIN0:PAD_N, :])
            pending_ag0 = None        # (ag0_tile, hb0_tile) awaiting collective
            qn = [0]

            def gather(half, c, k):
                nbg = CH_SIZES[c]
                g0 = CH_STARTS[c]
                T = T_A if half == 0 else T_B
                idx_s = idxA_s if half == 0 else idxB_s
                pool = gapool if half == 0 else gbpool
                n = nbg * T * 128
                g = pool.tile([128, nbg * T * D], dt.bfloat16,
                              tag="gA" if half == 0 else "gB")
                nc.gpsimd.dma_gather(
                    g[:].rearrange("p (t f) -> p t f", f=D),
                    srcs[half],
                    idx_s[:, g0 * T * 8:(g0 + nbg) * T * 8],
                    n, n, D, single_packet=False, queue_num=qn[0])
                qn[0] = (qn[0] + 1) % 4
                return g

            def build_S(half, c):
                nbg = CH_SIZES[c]
                g0 = CH_STARTS[c]
                T = T_A if half == 0 else T_B
                dv_s = dvA_s if half == 0 else dvB_s
                n = nbg * T
                S = spool.tile([128, n * 128], dt.bfloat16,
                               tag="SA" if half == 0 else "SB")
                nc.vector.tensor_tensor(
                    S[:].rearrange("p (t d) -> p t d", d=128),
                    iota_s[:].rearrange("p (o d) -> p o d", o=1)
                    .broadcast_to([128, n, 128]),
                    dv_s[:, g0 * T:(g0 + nbg) * T]
                    .rearrange("p (t o) -> p t o", o=1)
                    .broadcast_to([128, n, 128]),
                    mybir.AluOpType.is_equal)
                return S

            def compute_chunk(c, k, gA, gB, n_iters, ag0_t=None, ag1_t=None):
                nbg = CH_SIZES[c]
                g0 = CH_STARTS[c]
                SA = build_S(0, c)
                SB = build_S(1, c)
                ps = pp.tile([128, nbg * D], dt.float32, tag="ps")
                nmm = T_A + T_B
                for j in range(nbg):
                    mi = 0
                    for t in range(T_A):
                        i = j * T_A + t
                        nc.tensor.matmul(
                            ps[:, j * D:(j + 1) * D],
                            SA[:, i * 128:(i + 1) * 128],
                            gA[:, i * D:(i + 1) * D],
                            start=(mi == 0), stop=(mi == nmm - 1))
                        mi += 1
                    for t in range(T_B):
                        i = j * T_B + t
                        nc.tensor.matmul(
                            ps[:, j * D:(j + 1) * D],
                            SB[:, i * 128:(i + 1) * 128],
                            gB[:, i * D:(i + 1) * D],
                            start=(mi == 0), stop=(mi == nmm - 1))
                        mi += 1
                if k < n_iters:
                    # stage h_{k} for the AllGather on the ACT engine, then
                    # push this chunk's rows to the AG input buffer right away
                    # (small DMAs drain alongside the gather traffic, so the
                    # collective never waits on a big last-moment staging DMA).
                    # ACT is the ONLY psum consumer here: the out_acc update
                    # reads hstage once per hop, so DVE never waits on PE.
                    nc.scalar.copy(hstage[:, g0 * D:(g0 + nbg) * D], ps[:])
                    if c < NCH0:
                        dst = ag0_t[g0 * 128:(g0 + nbg) * 128, :]
                    else:
                        dst = ag1_t[(g0 - NB0) * 128:(g0 - NB0 + nbg) * 128, :]
                    nc.sync.dma_start(
                        dst.rearrange("(b p) f -> p b f", p=128),
                        hstage[:, g0 * D:(g0 + nbg) * D]
                        .rearrange("p (b f) -> p b f", f=D))
                else:
                    # last hop: h_K only feeds the output; accumulate in f32
                    # from psum directly (h_K dominates the result, keep it
                    # out of bf16).
                    nc.vector.scalar_tensor_tensor(
                        out_acc[:, g0 * D:(g0 + nbg) * D], ps[:],
                        wb[:, k:k + 1],
                        out_acc[:, g0 * D:(g0 + nbg) * D],
                        mybir.AluOpType.mult, mybir.AluOpType.add)

            for k in range(1, n_iters + 1):
                # Window-phased schedule: buf1's dst blocks (chunks 6..12) are
                # computed first (their B-halves read hb1^{k-1}, staged
                # mid-previous-hop), so AG1^k can fly mid-hop; buf0's blocks
                # (chunks 0..5) compute second, staged at hop end, and AG0^k's
                # collective is deferred into hop k+1 once staging completes.
                gB_tiles = {}
                hb0_new = None
                hb1_new = None
                ag0 = ag1 = None
                if k < n_iters:
                    ag0 = dram.tile([SPLIT_L, D], dt.bfloat16, tag="ag0")
                    ag1 = dram.tile([LPC1, D], dt.bfloat16, tag="ag1")
                # phase 1: B5..B9, B0, B1 (all read hb1^{k-1}); the AG0^{k-1}
                # collective is slotted in early, once its staging drained.
                for i, c in enumerate([*range(NCH0, NCH), 0, 1]):
                    gB_tiles[c] = gather(1, c, k)
                    if i == 1 and pending_ag0 is not None:
                        ag0_t, hb0_t = pending_ag0
                        nc.gpsimd.collective_compute(
                            "AllGather", mybir.AluOpType.bypass,
                            replica_groups=[list(range(CORES))],
                            ins=[ag0_t.opt()], outs=[hb0_t.opt()])
                        pending_ag0 = None
                # phase 2: A5..A9, computing chunks 5..9 as pairs complete
                for c in range(NCH0, NCH):
                    gA = gather(0, c, k)
                    compute_chunk(c, k, gA, gB_tiles.pop(c), n_iters, ag0, ag1)
                if k < n_iters:
                    # AG1^k: its chunks were staged through phase 2; issue now
                    hb1_new = dram.tile([WIN1, D], dt.bfloat16, tag="hb1")
                    nc.gpsimd.collective_compute(
                        "AllGather", mybir.AluOpType.bypass,
                        replica_groups=[list(range(CORES))],
                        ins=[ag1.opt()], outs=[hb1_new.opt()])
                # phase 3: remaining B's woven between A0..A4 + computes
                for c in range(NCH0):
                    gA = gather(0, c, k)
                    if c >= 2:
                        gB_tiles[c] = gather(1, c, k)
                    compute_chunk(c, k, gA, gB_tiles.pop(c), n_iters, ag0, ag1)
                if k < n_iters:
                    # single fused out_acc += w_k * h_k from the bf16 staging
                    # (negligible extra rounding for k < K; h_K stays f32)
                    nc.vector.scalar_tensor_tensor(
                        out_acc[:], hstage[:], wb[:, k:k + 1], out_acc[:],
                        mybir.AluOpType.mult, mybir.AluOpType.add)
                    hb0_new = dram.tile([WIN0, D], dt.bfloat16, tag="hb0")
                    pending_ag0 = (ag0, hb0_new)
                    srcs = (hb0_new[:], hb1_new[:])
            nc.sync.dma_start(
                outp.ap().rearrange("(b p) f -> p b f", p=128),
                out_acc[:].rearrange("p (b f) -> p b f", f=D))
    nc.compile()
    return nc


def _maybe_install_trace_hook():
    import types
    import antenv
    if "antenv.axon_hooks" in sys.modules:
        return
    hooks = types.ModuleType("antenv.axon_hooks")
    hooks._hook = None
    hooks.set_axon_ntff_profile_hook = lambda h: setattr(hooks, "_hook", h)
    hooks.get_axon_ntff_profile_hook = lambda: hooks._hook
    sys.modules["antenv.axon_hooks"] = hooks
    antenv.axon_hooks = hooks
    try:
        from trn_agent_boot.trn_boot import _ntff_profile_via_ctypes
        hooks.set_axon_ntff_profile_hook(
            _ntff_profile_via_ctypes("/opt/axon/libaxon_pjrt.so"))
    except Exception:
        pass


def kernel(x, att, edge_index):
    global LAST_EXEC_NS, LAST_RESULT
    x = np.asarray(x)
    att = np.asarray(att)
    edge_index = np.asarray(edge_index)
    n_iters = int(os.environ.get("DAGNN_K", K))
    T_A, T_B, in_maps = _host_prep(x, att, edge_index)
    key = (T_A, T_B, n_iters)
    if key not in _nc_cache:
        _nc_cache[key] = _build(T_A, T_B, n_iters)
    nc = _nc_cache[key]
    trace = os.environ.get("BASS_KERNEL_TRACE", "0") == "1"
    if trace:
        _maybe_install_trace_hook()
    res = run_bass_kernel_spmd(nc, in_maps, core_ids=list(range(CORES)),
                               trace=trace)
    LAST_RESULT = res
    LAST_EXEC_NS = res.exec_time_ns
    out = np.concatenate([res.results[c]["out"] for c in range(CORES)], axis=0)
    return np.ascontiguousarray(out[:N_NODES]).astype(np.float32)


# revision 17
# speedup vs baseline: 1.0546x; 1.0546x over previous
"""DAGNN K-hop propagation (out = sum_k softmax(att)[k] * A^k x) on 8 TRN2 cores.

Sharding: nodes are split into 8 contiguous shards of 6272 rows (49 blocks of
128). Edges are partitioned by destination-owner core, so each core's
segment-sum is local; per hop, the 8 bf16 h-shards are exchanged with two
AllGathers (split by local row range so each gather window fits int16
indices).

v2 restructure (Q7-saturating schedule): the GpSimd engine (which serially
generates every gather's DMA descriptors) is the bottleneck, so the per-hop
instruction stream is ordered to keep it busy end-to-end:
  - B-half gathers lag A-half gathers by LAG chunks, so the hop's first
    gathers (A, reading the buf0 window) never wait on the late AllGather
    (buf1), and the AG1 collective instruction - which runs on the GpSimd
    queue and blocks it while waiting for its staging DMA - is issued early
    in the NEXT hop when its input is already staged.
  - AG0 covers blocks 0..23 (staged mid-hop after chunk 3's results), its
    collective issued late in the hop when staging has completed.
  - PSUM->SBUF h-staging copies run on the idle Scalar/ACT engine; out_acc
    updates are a single fused DVE op.

Per hop, per core: dma_gather pulls the ~88k source-row slots (bf16, 256B
each) from DRAM into SBUF in 128-edge tiles laid out per (dst-block,
src-window) segment; a one-hot segment matrix S per tile (broadcast-AP
vector-engine is_equal per chunk-half) is matmul'd with the gathered tile on
the tensor engine, accumulating each dst-block's segment sum in PSUM. Pad
slots gather row 0 with dst value -1, which maps to a zero one-hot column.
"""
import os
import sys

sys.path.insert(0, "/opt/trn_rl_repo")

import numpy as np
import ml_dtypes

import concourse.bacc as bacc
import concourse.mybir as mybir
from concourse import tile
from concourse.bass_utils import run_bass_kernel_spmd

N_NODES, N_EDGES, D, K = 50000, 625000, 128, 10
CORES, NB = 8, 49
NPC = NB * 128              # 6272 nodes per core
PAD_N = CORES * NPC         # 50176
SPLIT_L = 3200              # local-row split: rows [0,3200) -> buf0
NB0 = SPLIT_L // 128        # 24 blocks in buf0 half
WIN0 = CORES * SPLIT_L      # 24576 rows in buf0
WIN1 = PAD_N - WIN0         # 25600 rows in buf1
LPC1 = NPC - SPLIT_L        # 3200 rows/core in buf1
CH_SIZES = [5] * 9 + [4]
CH_STARTS = [5 * i for i in range(9)] + [45]
NCH = len(CH_SIZES)
NCH0 = 5                    # chunks 0..4 cover buf0's blocks (0..24)
BF16 = ml_dtypes.bfloat16

LAST_EXEC_NS = None         # set when BASS_KERNEL_TRACE=1
LAST_RESULT = None

_nc_cache = {}


def _host_prep(x, att, edge_index):
    src = np.asarray(edge_index[0], dtype=np.int64)
    dst = np.asarray(edge_index[1], dtype=np.int64)
    # src row position in the split h-table layout
    c0 = src // NPC
    l0 = src % NPC
    half = (l0 >= SPLIT_L).astype(np.int64)
    rowidx = np.where(half == 0, c0 * SPLIT_L + l0,
                      c0 * LPC1 + (l0 - SPLIT_L))
    gblk = dst >> 7                                  # global 128-block, 0..390
    seg = gblk * 2 + half
    order = np.argsort(seg, kind="stable")
    seg_s = seg[order]
    counts = np.bincount(seg, minlength=2 * (NB * CORES))
    T_A = int(np.ceil(counts[0::2].max() / 128))
    T_B = int(np.ceil(counts[1::2].max() / 128))
    LA, LB = NB * T_A * 128, NB * T_B * 128

    starts = np.zeros(2 * NB * CORES + 1, np.int64)
    np.cumsum(counts, out=starts[1:])
    rank = np.arange(N_EDGES) - starts[seg_s]
    row_s, dst_s = rowidx[order], dst[order]
    core = gblk[order] // NB
    b = gblk[order] % NB
    mA = seg_s % 2 == 0

    idxA = np.full((CORES, LA), 0, np.int16)
    dvA = np.full((CORES, LA), -1.0, np.float32)
    idxB = np.full((CORES, LB), 0, np.int16)
    dvB = np.full((CORES, LB), -1.0, np.float32)
    sA = b[mA] * (T_A * 128) + rank[mA]
    idxA[core[mA], sA] = row_s[mA].astype(np.int16)
    dvA[core[mA], sA] = (dst_s[mA] & 127).astype(np.float32)
    mB = ~mA
    sB = b[mB] * (T_B * 128) + rank[mB]
    idxB[core[mB], sB] = row_s[mB].astype(np.int16)
    dvB[core[mB], sB] = (dst_s[mB] & 127).astype(np.float32)

    # h0 in the split layout
    x_pad = np.zeros((PAD_N, D), np.float32)
    x_pad[:N_NODES] = np.asarray(x, np.float32)
    xr = x_pad.reshape(CORES, NPC, D)
    h0 = np.concatenate(
        [xr[:, :SPLIT_L].reshape(-1, D), xr[:, SPLIT_L:].reshape(-1, D)],
        axis=0).astype(BF16)
    attw = np.ascontiguousarray(att, dtype=np.float32).reshape(1, K + 1)

    in_maps = []
    for c in range(CORES):
        in_maps.append({
            "h0": h0,
            "xsh": np.ascontiguousarray(x_pad[c * NPC:(c + 1) * NPC]),
            "attw": attw,
            # idx i lives at [i % 16, i // 16], replicated to 8 groups of 16
            # partitions for the Q7 cores.
            "idxA": np.ascontiguousarray(np.tile(idxA[c].reshape(-1, 16).T, (8, 1))),
            "idxB": np.ascontiguousarray(np.tile(idxB[c].reshape(-1, 16).T, (8, 1))),
            # dstv column t holds tile t's per-edge dst-within-block values.
            "dvA": np.ascontiguousarray(dvA[c].reshape(-1, 128).T),
            "dvB": np.ascontiguousarray(dvB[c].reshape(-1, 128).T),
        })
    return T_A, T_B, in_maps


def _build(T_A, T_B, n_iters=K):
    dt = mybir.dt
    nc = bacc.Bacc("TRN2", target_bir_lowering=False, debug=False,
                   num_devices=CORES, num_swdge_queues=4)
    h0 = nc.dram_tensor("h0", [PAD_N, D], dt.bfloat16, kind="ExternalInput")
    xsh = nc.dram_tensor("xsh", [NPC, D], dt.float32, kind="ExternalInput")
    attw = nc.dram_tensor("attw", [1, K + 1], dt.float32, kind="ExternalInput")
    idxA = nc.dram_tensor("idxA", [128, NB * T_A * 8], dt.int16, kind="ExternalInput")
    idxB = nc.dram_tensor("idxB", [128, NB * T_B * 8], dt.int16, kind="ExternalInput")
    dvA = nc.dram_tensor("dvA", [128, NB * T_A], dt.float32, kind="ExternalInput")
    dvB = nc.dram_tensor("dvB", [128, NB * T_B], dt.float32, kind="ExternalInput")
    outp = nc.dram_tensor("out", [NPC, D], dt.float32, kind="ExternalOutput")
    iota_c = nc.inline_tensor(
        np.ascontiguousarray(
            np.broadcast_to(np.arange(128, dtype=np.float32), (128, 128))
        ),
        name="iota",
    )

    with tile.TileContext(nc) as tc:
        with (
            tc.tile_pool(name="pers", bufs=1) as pers,
            tc.tile_pool(name="gapool", bufs=5) as gapool,
            tc.tile_pool(name="gbpool", bufs=7) as gbpool,
            tc.tile_pool(name="spool", bufs=2) as spool,
            tc.tile_pool(name="pp", bufs=4, space="PSUM") as pp,
            tc.tile_pool(name="tpool", bufs=2) as tpool,
            tc.tile_pool(name="dram", bufs=2, space="DRAM") as dram,
        ):
            idxA_s = pers.tile([128, NB * T_A * 8], dt.int16)
            nc.sync.dma_start(idxA_s[:], idxA[:])
            idxB_s = pers.tile([128, NB * T_B * 8], dt.int16)
            nc.sync.dma_start(idxB_s[:], idxB[:])
            dvA_s = pers.tile([128, NB * T_A], dt.float32)
            nc.sync.dma_start(dvA_s[:], dvA[:])
            dvB_s = pers.tile([128, NB * T_B], dt.float32)
            nc.sync.dma_start(dvB_s[:], dvB[:])
            iota_s = pers.tile([128, 128], dt.float32)
            nc.sync.dma_start(iota_s[:], iota_c[:])

            # w = softmax(att) on partition 0, broadcast to all partitions.
            att_s = pers.tile([1, K + 1], dt.float32)
            nc.sync.dma_start(att_s[:], attw[:])
            wexp = pers.tile([1, K + 1], dt.float32)
            nc.scalar.activation(wexp[:], att_s[:],
                                 mybir.ActivationFunctionType.Exp)
            wsum = pers.tile([1, 1], dt.float32)
            nc.vector.tensor_reduce(wsum[:], wexp[:], mybir.AxisListType.X,
                                    mybir.AluOpType.add)
            wrec = pers.tile([1, 1], dt.float32)
            nc.vector.reciprocal(wrec[:], wsum[:])
            wnorm = pers.tile([1, K + 1], dt.float32)
            nc.vector.tensor_scalar_mul(wnorm[:], wexp[:], wrec[:])
            wb = pers.tile([128, K + 1], dt.float32)
            nc.gpsimd.partition_broadcast(wb[:], wnorm[:])

            # out_acc[p, b*D:...] accumulates node (b*128+p)'s output row.
            out_acc = pers.tile([128, NB * D], dt.float32)
            hstage = pers.tile([128, NB * D], dt.bfloat16)
            for ci in range(NCH):
                g0, nbg = CH_STARTS[ci], CH_SIZES[ci]
                xc = tpool.tile([128, nbg * D], dt.float32, tag="tmp")
                nc.sync.dma_start(
                    xc[:].rearrange("p (b f) -> p b f", f=D),
                    xsh.ap()[g0 * 128:(g0 + nbg) * 128, :]
                    .rearrange("(b p) f -> p b f", p=128))
                # init on the ACT engine: out_acc = w0 * x
                nc.scalar.activation(
                    out_acc[:, g0 * D:(g0 + nbg) * D], xc[:],
                    mybir.ActivationFunctionType.Copy, scale=wb[:, 0:1])

            srcs = (h0.ap()[0:WIN0, :], h0.ap()[WIN0:PAD_N, :])
            pending_ag0 = None        # (ag0_tile, hb0_tile) awaiting collective
            qn = [0]

            def gather(half, c, k):
                nbg = CH_SIZES[c]
                g0 = CH_STARTS[c]
                T = T_A if half == 0 else T_B
                idx_s = idxA_s if half == 0 else idxB_s
                pool = gapool if half == 0 else gbpool
                n = nbg * T * 128
                g = pool.tile([128, nbg * T * D], dt.bfloat16,
                              tag="gA" if half == 0 else "gB")
                nc.gpsimd.dma_gather(
                    g[:].rearrange("p (t f) -> p t f", f=D),
                    srcs[half],
                    idx_s[:, g0 * T * 8:(g0 + nbg) * T * 8],
                    n, n, D, single_packet=False, queue_num=qn[0])
                qn[0] = (qn[0] + 1) % 3
                return g

            def build_S(half, c):
                nbg = CH_SIZES[c]
                g0 = CH_STARTS[c]
                T = T_A if half == 0 else T_B
                dv_s = dvA_s if half == 0 else dvB_s
                n = nbg * T
                S = spool.tile([128, n * 128], dt.bfloat16,
                               tag="SA" if half == 0 else "SB")
                nc.vector.tensor_tensor(
                    S[:].rearrange("p (t d) -> p t d", d=128),
                    iota_s[:].rearrange("p (o d) -> p o d", o=1)
                    .broadcast_to([128, n, 128]),
                    dv_s[:, g0 * T:(g0 + nbg) * T]
                    .rearrange("p (t o) -> p t o", o=1)
                    .broadcast_to([128, n, 128]),
                    mybir.AluOpType.is_equal)
                return S

            def compute_chunk(c, k, gA, gB, n_iters, ag0_t=None, ag1_t=None):
                nbg = CH_SIZES[c]
                g0 = CH_STARTS[c]
                SA = build_S(0, c)
                SB = build_S(1, c)
                ps = pp.tile([128, nbg * D], dt.float32, tag="ps")
                nmm = T_A + T_B
                for j in range(nbg):
                    mi = 0
                    for t in range(T_A):
                        i = j * T_A + t
                        nc.tensor.matmul(
                            ps[:, j * D:(j + 1) * D],
                            SA[:, i * 128:(i + 1) * 128],
                            gA[:, i * D:(i + 1) * D],
                            start=(mi == 0), stop=(mi == nmm - 1))
                        mi += 1
                    for t in range(T_B):
                        i = j * T_B + t
                        nc.tensor.matmul(
                            ps[:, j * D:(j + 1) * D],
                            SB[:, i * 128:(i + 1) * 128],
                            gB[:, i * D:(i + 1) * D],
                            start=(mi == 0), stop=(mi == nmm - 1))
                        mi += 1
                if k < n_iters:
                    # stage h_{k} for the AllGather on the ACT engine, then
                    # push this chunk's rows to the AG input buffer right away
                    # (small DMAs drain alongside the gather traffic, so the
                    # collective never waits on a big last-moment staging DMA).
                    # ACT is the ONLY psum consumer here: the out_acc update
                    # reads hstage once per hop, so DVE never waits on PE.
                    nc.scalar.copy(hstage[:, g0 * D:(g0 + nbg) * D], ps[:])
                    if c < NCH0:
                        dst = ag0_t[g0 * 128:(g0 + nbg) * 128, :]
                    else:
                        dst = ag1_t[(g0 - NB0) * 128:(g0 - NB0 + nbg) * 128, :]
                    nc.sync.dma_start(
                        dst.rearrange("(b p) f -> p b f", p=128),
                        hstage[:, g0 * D:(g0 + nbg) * D]
                        .rearrange("p (b f) -> p b f", f=D))
                else:
                    # last hop: h_K only feeds the output; accumulate in f32
                    # from psum directly (h_K dominates the result, keep it
                    # out of bf16).
                    nc.vector.scalar_tensor_tensor(
                        out_acc[:, g0 * D:(g0 + nbg) * D], ps[:],
                        wb[:, k:k + 1],
                        out_acc[:, g0 * D:(g0 + nbg) * D],
                        mybir.AluOpType.mult, mybir.AluOpType.add)

            for k in range(1, n_iters + 1):
                # Window-phased schedule: buf1's dst blocks (chunks 6..12) are
                # computed first (their B-halves read hb1^{k-1}, staged
                # mid-previous-hop), so AG1^k can fly mid-hop; buf0's blocks
                # (chunks 0..5) compute second, staged at hop end, and AG0^k's
                # collective is deferred into hop k+1 once staging completes.
                gB_tiles = {}
                hb0_new = None
                hb1_new = None
                ag0 = ag1 = None
                if k < n_iters:
                    ag0 = dram.tile([SPLIT_L, D], dt.bfloat16, tag="ag0")
                    ag1 = dram.tile([LPC1, D], dt.bfloat16, tag="ag1")
                # phase 1: B5..B9 (hb1^{k-1} was ready mid-previous hop)
                for i, c in enumerate(range(NCH0, NCH)):
                    gB_tiles[c] = gather(1, c, k)
                    if i == 1 and pending_ag0 is not None:
                        ag0_t, hb0_t = pending_ag0
                        nc.gpsimd.collective_compute(
                            "AllGather", mybir.AluOpType.bypass,
                            replica_groups=[list(range(CORES))],
                            ins=[ag0_t.opt()], outs=[hb0_t.opt()])
                        pending_ag0 = None
                # phase 2: A5..A9, computing chunks 5..9 as pairs complete
                for c in range(NCH0, NCH):
                    gA = gather(0, c, k)
                    compute_chunk(c, k, gA, gB_tiles.pop(c), n_iters, ag0, ag1)
                if k < n_iters:
                    # AG1^k: its chunks were staged through phase 2; issue now
                    hb1_new = dram.tile([WIN1, D], dt.bfloat16, tag="hb1")
                    nc.gpsimd.collective_compute(
                        "AllGather", mybir.AluOpType.bypass,
                        replica_groups=[list(range(CORES))],
                        ins=[ag1.opt()], outs=[hb1_new.opt()])
                # phase 3: A0,B0 .. A4,B4, computing chunks 0..4
                for c in range(NCH0):
                    gA = gather(0, c, k)
                    gB_tiles[c] = gather(1, c, k)
                    compute_chunk(c, k, gA, gB_tiles.pop(c), n_iters, ag0, ag1)
                if k < n_iters:
                    # single fused out_acc += w_k * h_k from the bf16 staging
                    # (negligible extra rounding for k < K; h_K stays f32)
                    nc.vector.scalar_tensor_tensor(
                        out_acc[:], hstage[:], wb[:, k:k + 1], out_acc[:],
                        mybir.AluOpType.mult, mybir.AluOpType.add)
                    hb0_new = dram.tile([WIN0, D], dt.bfloat16, tag="hb0")
                    pending_ag0 = (ag0, hb0_new)
                    srcs = (hb0_new[:], hb1_new[:])
            nc.sync.dma_start(
                outp.ap().rearrange("(b p) f -> p b f", p=128),
                out_acc[:].rearrange("p (b f) -> p b f", f=D))
    nc.compile()
    return nc


def _maybe_install_trace_hook():
    import types
    import antenv
    if "antenv.axon_hooks" in sys.modules:
        return
    hooks = types.ModuleType("antenv.axon_hooks")
    hooks._hook = None
    hooks.set_axon_ntff_profile_hook = lambda h: setattr(hooks, "_hook", h)
    hooks.get_axon_ntff_profile_hook = lambda: hooks._hook
    sys.modules["antenv.axon_hooks"] = hooks
    antenv.axon_hooks = hooks
    try:
        from trn_agent_boot.trn_boot import _ntff_profile_via_ctypes
        hooks.set_axon_ntff_profile_hook(
            _ntff_profile_via_ctypes("/opt/axon/libaxon_pjrt.so"))
    except Exception:
        pass


def kernel(x, att, edge_index):
    global LAST_EXEC_NS, LAST_RESULT
    x = np.asarray(x)
    att = np.asarray(att)
    edge_index = np.asarray(edge_index)
    n_iters = int(os.environ.get("DAGNN_K", K))
    T_A, T_B, in_maps = _host_prep(x, att, edge_index)
    key = (T_A, T_B, n_iters)
    if key not in _nc_cache:
        _nc_cache[key] = _build(T_A, T_B, n_iters)
    nc = _nc_cache[key]
    trace = os.environ.get("BASS_KERNEL_TRACE", "0") == "1"
    if trace:
        _maybe_install_trace_hook()
    res = run_bass_kernel_spmd(nc, in_maps, core_ids=list(range(CORES)),
                               trace=trace)
    LAST_RESULT = res
    LAST_EXEC_NS = res.exec_time_ns
    out = np.concatenate([res.results[c]["out"] for c in range(CORES)], axis=0)
    return np.ascontiguousarray(out[:N_NODES]).astype(np.float32)
